# revision 14
# baseline (speedup 1.0000x reference)
"""Bass/Tile TRN2 kernel for BasicAttention.

att = softmax(tanh(hidden @ W_h.T + p_att_feats) @ W_alpha + mask) @ att_feats

Shapes: B=64, N=2048, H=1024, A=512. Data-parallel over batch across 8
NeuronCores (8 batches per core); weights replicated; no collectives.

Layout: region index n maps to (partition p, column c) as n = p*16 + c so
every p_att/att_feats DMA is a long contiguous per-partition read and the
mask tile is a natural row-major reshape.

Per-core dataflow (memory-bound: ~96MB HBM reads/core):
  host: pass W_h.T, hidden.T and a pre-broadcast bf16 W_alpha (layout-only
        transforms) so no PE transposes are needed on device.
  setup: w_h = hidden @ W_h.T (PE) -> per-batch partition-broadcast of
         w_h rows via a DRAM round-trip 0-stride DMA.
  per batch b (software-pipelined, p_att phase leads att_feats phase):
    p_att stream [128,8,512]: DVE add (w_h bcast) -> ACT tanh (bf16)
      -> DVE scalar_tensor_tensor vs W_alpha (accum) -> scores[128,16]
    scores: + mask, ACT exp (accum rowsum, f32r out), PE total-sum,
      DVE reciprocal
    att_feats stream [128,2,1024] f32r: PE matmuls (attn col stationary)
      accumulating att[1,1024] in PSUM -> DVE scale by 1/sum -> out.
"""

import numpy as np

B, N, H, A = 64, 2048, 1024, 512
NCORES = 8
BLOC = B // NCORES  # batches per core

P = 128
NT = N // P            # 16 n-columns per partition
PATT_SUP = 8           # columns per p_att supertile (2 DMAs per batch)
AF_SUP = 8             # columns per att_feats supertile (2 DMAs per batch)

_NC_CACHE = {}


def _free_bcast(bass_mod, ap, repeat):
    """[P, F] AP -> [P, repeat, F] AP with 0-stride middle dim."""
    return bass_mod.AP(
        tensor=ap.tensor,
        offset=ap.offset,
        ap=[ap.ap[0], [0, repeat], *ap.ap[1:]],
    )


def _build_nc():
    import concourse.bass as bass
    import concourse.mybir as mybir
    import concourse.tile as tile
    from concourse import bacc

    dt = mybir.dt
    f32, f32r, bf16 = dt.float32, dt.float32r, dt.bfloat16
    AF = mybir.ActivationFunctionType
    OP = mybir.AluOpType

    nc = bacc.Bacc("TRN2", target_bir_lowering=False, debug=False,
                   num_devices=NCORES)

    hsT = nc.dram_tensor("hidden_T", [H, BLOC], f32, kind="ExternalInput").ap()
    af = nc.dram_tensor("att_feats", [BLOC, N, H], f32r, kind="ExternalInput").ap()
    pa = nc.dram_tensor("p_att_feats", [BLOC, N, A], f32, kind="ExternalInput").ap()
    am = nc.dram_tensor("att_masks", [BLOC, N], f32, kind="ExternalInput").ap()
    whT = nc.dram_tensor("W_hT", [H, A], f32, kind="ExternalInput").ap()
    wab = nc.dram_tensor("W_alpha_b", [P, A], bf16, kind="ExternalInput").ap()
    out = nc.dram_tensor("att_out", [BLOC, H], f32, kind="ExternalOutput").ap()

    with tile.TileContext(nc) as tc:
        with (
            tc.tile_pool(name="consts", bufs=1) as consts,
            tc.tile_pool(name="patt", bufs=3) as patt_pool,
            tc.tile_pool(name="alpha", bufs=2) as alpha_pool,
            tc.tile_pool(name="afp", bufs=2) as af_pool,
            tc.tile_pool(name="small", bufs=2) as small,
            tc.tile_pool(name="psmisc", bufs=2, space="PSUM") as psmisc,
            tc.tile_pool(name="psatt", bufs=6, space="PSUM") as psatt,
        ):
            # ---------------- setup ----------------
            ones_col = consts.tile([P, 1], f32)
            nc.vector.memset(ones_col, 1.0)

            whT_sb = []
            for hc in range(H // P):  # 8 tiles [128h, 512a], contiguous rows
                t = consts.tile([P, A], f32, name=f"whT{hc}", tag=f"whT{hc}")
                nc.sync.dma_start(out=t, in_=whT[hc * P:(hc + 1) * P, :])
                whT_sb.append(t)
            hidT_sb = []
            for hc in range(H // P):  # 8 tiles [128h, 8b]
                t = consts.tile([P, BLOC], f32, name=f"hidT{hc}", tag=f"hidT{hc}")
                nc.sync.dma_start(out=t, in_=hsT[hc * P:(hc + 1) * P, :])
                hidT_sb.append(t)
            wa_bf = consts.tile([P, A], bf16)
            nc.sync.dma_start(out=wa_bf, in_=wab[:, :])

            # w_h = hidden @ W_h.T : [8, 512]
            wh_ps = psmisc.tile([BLOC, A], f32, tag="mm")
            for hc in range(H // P):
                nc.tensor.matmul(wh_ps, lhsT=hidT_sb[hc], rhs=whT_sb[hc],
                                 start=(hc == 0), stop=(hc == H // P - 1))
            whall_sb = consts.tile([BLOC, A], f32)
            nc.vector.tensor_copy(whall_sb, wh_ps)

            # per-batch w_h row broadcast to [128, 512] f32 via a DRAM
            # round-trip with a 0-stride partition AP (setup-only, ~2MB)
            whall_dram = nc.dram_tensor("whall_scratch", [BLOC, A], f32).ap()
            nc.sync.dma_start(out=whall_dram, in_=whall_sb)
            whb = []
            for b in range(BLOC):
                t = consts.tile([P, A], f32, name=f"whb{b}", tag=f"whb{b}")
                row = whall_dram[b:b + 1, :]
                src = bass.AP(tensor=row.tensor, offset=row.offset,
                              ap=[[0, P], row.ap[1]])
                nc.sync.dma_start(out=t, in_=src)
                whb.append(t)

            # ---------------- main loop (software-pipelined) ----------------
            # n = p*NT + c everywhere below.
            pa_r = [pa[b, :, :].rearrange("(p c) a -> p c a", c=NT)
                    for b in range(BLOC)]
            af_r = [af[b, :, :].rearrange("(p c) h -> p c h", c=NT)
                    for b in range(BLOC)]

            def patt_phase(b):
                scores = small.tile([P, NT], f32, tag="scores",
                                    name=f"scores{b}")
                for st in range(NT // PATT_SUP):  # 2 supertiles
                    pt = patt_pool.tile([P, PATT_SUP, A], f32, tag="patt",
                                        name=f"patt{b}_{st}")
                    nc.sync.dma_start(
                        out=pt,
                        in_=pa_r[b][:, st * PATT_SUP:(st + 1) * PATT_SUP, :],
                    )
                    whb_b = _free_bcast(bass, whb[b][:, :], PATT_SUP)
                    nc.vector.tensor_tensor(out=pt, in0=pt, in1=whb_b, op=OP.add)
                    ab = alpha_pool.tile([P, PATT_SUP, A], bf16, tag="alpha",
                                         name=f"alpha{b}_{st}")
                    nc.scalar.activation(ab, pt, AF.Tanh)
                    for c in range(PATT_SUP):
                        col = st * PATT_SUP + c
                        # out = (ab * 1) * wa ; accum_out = row-sum -> scores
                        nc.vector.scalar_tensor_tensor(
                            out=ab[:, c, :], in0=ab[:, c, :], scalar=1.0,
                            in1=wa_bf, op0=OP.mult, op1=OP.mult,
                            accum_out=scores[:, col:col + 1],
                        )

                masks = small.tile([P, NT], f32, tag="masks", name=f"masks{b}")
                nc.sync.dma_start(
                    out=masks, in_=am[b, :].rearrange("(p c) -> p c", c=NT))
                nc.vector.tensor_tensor(out=scores, in0=scores, in1=masks,
                                        op=OP.add)

                expt = small.tile([P, NT], f32r, tag="expt", name=f"expt{b}")
                rowsum = small.tile([P, 1], f32, tag="rowsum", name=f"rowsum{b}")
                nc.scalar.activation(expt, scores, AF.Exp, accum_out=rowsum)

                sum_ps = psmisc.tile([1, 1], f32, tag="mm", name=f"sum_ps{b}")
                nc.tensor.matmul(sum_ps, lhsT=rowsum, rhs=ones_col,
                                 start=True, stop=True)
                inv = small.tile([1, 1], f32, tag="inv", name=f"inv{b}")
                nc.vector.reciprocal(inv, sum_ps)
                return expt, inv

            def af_phase(b, expt, inv):
                att_lo = psatt.tile([1, A], f32, tag="att", name=f"attlo{b}")
                att_hi = psatt.tile([1, A], f32, tag="att", name=f"atthi{b}")
                for st2 in range(NT // AF_SUP):
                    aft = af_pool.tile([P, AF_SUP, H], f32r, tag="af",
                                       name=f"af{b}_{st2}")
                    nc.sync.dma_start(
                        out=aft,
                        in_=af_r[b][:, st2 * AF_SUP:(st2 + 1) * AF_SUP, :],
                    )
                    for c in range(AF_SUP):
                        t = st2 * AF_SUP + c
                        lhs = expt[:, t:t + 1]
                        nc.tensor.matmul(att_lo, lhsT=lhs,
                                         rhs=aft[:, c, 0:A],
                                         start=(t == 0), stop=(t == NT - 1))
                        nc.tensor.matmul(att_hi, lhsT=lhs,
                                         rhs=aft[:, c, A:H],
                                         start=(t == 0), stop=(t == NT - 1))

                att_row = small.tile([1, H], f32, tag="attrow",
                                     name=f"attrow{b}")
                nc.vector.tensor_scalar_mul(att_row[:, 0:A], att_lo, inv)
                nc.vector.tensor_scalar_mul(att_row[:, A:H], att_hi, inv)
                nc.sync.dma_start(out=out[b:b + 1, :], in_=att_row)

            state = {}
            for b in range(BLOC):
                state[b] = patt_phase(b)
                if b >= 1:
                    af_phase(b - 1, *state.pop(b - 1))
            af_phase(BLOC - 1, *state.pop(BLOC - 1))

    nc.compile()
    return nc


def _get_nc():
    if "nc" not in _NC_CACHE:
        _NC_CACHE["nc"] = _build_nc()
    return _NC_CACHE["nc"]


def kernel(hidden_states, att_feats, p_att_feats, att_masks, W_h, W_alpha):
    import ml_dtypes
    from concourse.bass_utils import run_bass_kernel_spmd

    nc = _get_nc()
    hidden_states = np.ascontiguousarray(hidden_states, dtype=np.float32)
    att_feats = np.ascontiguousarray(att_feats, dtype=np.float32)
    p_att_feats = np.ascontiguousarray(p_att_feats, dtype=np.float32)
    att_masks = np.ascontiguousarray(att_masks, dtype=np.float32)
    W_h = np.ascontiguousarray(W_h, dtype=np.float32)
    W_alpha = np.asarray(W_alpha, dtype=np.float32).reshape(1, A)

    whT = np.ascontiguousarray(W_h.T)                       # [H, A]
    wab = np.ascontiguousarray(
        np.broadcast_to(W_alpha, (P, A))).astype(ml_dtypes.bfloat16)

    in_maps = []
    for i in range(NCORES):
        s = slice(i * BLOC, (i + 1) * BLOC)
        in_maps.append({
            "hidden_T": np.ascontiguousarray(hidden_states[s].T),
            "att_feats": att_feats[s],
            "p_att_feats": p_att_feats[s],
            "att_masks": att_masks[s],
            "W_hT": whT,
            "W_alpha_b": wab,
        })

    global _LAST_IN_MAPS
    _LAST_IN_MAPS = in_maps
    res = run_bass_kernel_spmd(nc, in_maps, core_ids=list(range(NCORES)))
    return np.concatenate(
        [res.results[i]["att_out"] for i in range(NCORES)], axis=0
    ).astype(np.float32)


_LAST_IN_MAPS = None


# revision 15
# speedup vs baseline: 1.0957x; 1.0957x over previous
"""Bass/Tile TRN2 kernel for BasicAttention.

att = softmax(tanh(hidden @ W_h.T + p_att_feats) @ W_alpha + mask) @ att_feats

Shapes: B=64, N=2048, H=1024, A=512. Data-parallel over batch across 8
NeuronCores (8 batches per core); weights replicated; no collectives.

Layout: region index n maps to (partition p, column c) as n = p*16 + c so
every p_att/att_feats DMA is a long contiguous per-partition read and the
mask tile is a natural row-major reshape.

Per-core dataflow (memory-bound: ~96MB HBM reads/core):
  host: pass W_h.T, hidden.T and a pre-broadcast bf16 W_alpha (layout-only
        transforms) so no PE transposes are needed on device.
  setup: w_h = hidden @ W_h.T (PE) -> per-batch partition-broadcast of
         w_h rows via a DRAM round-trip 0-stride DMA.
  per batch b (software-pipelined, p_att phase leads att_feats phase):
    p_att stream [128,8,512]: DVE add (w_h bcast) -> ACT tanh (bf16)
      -> DVE scalar_tensor_tensor vs W_alpha (accum) -> scores[128,16]
    scores: + mask, ACT exp (accum rowsum, f32r out), PE total-sum,
      DVE reciprocal
    att_feats stream [128,2,1024] f32r: PE matmuls (attn col stationary)
      accumulating att[1,1024] in PSUM -> DVE scale by 1/sum -> out.
"""

import numpy as np

B, N, H, A = 64, 2048, 1024, 512
NCORES = 8
BLOC = B // NCORES  # batches per core

P = 128
NT = N // P            # 16 n-columns per partition
PATT_SUP = 8           # columns per p_att supertile (2 DMAs per batch)
AF_SUP = 4             # columns per att_feats supertile (4 DMAs per batch)

_NC_CACHE = {}


def _free_bcast(bass_mod, ap, repeat):
    """[P, F] AP -> [P, repeat, F] AP with 0-stride middle dim."""
    return bass_mod.AP(
        tensor=ap.tensor,
        offset=ap.offset,
        ap=[ap.ap[0], [0, repeat], *ap.ap[1:]],
    )


def _build_nc():
    import concourse.bass as bass
    import concourse.mybir as mybir
    import concourse.tile as tile
    from concourse import bacc

    dt = mybir.dt
    f32, f32r, bf16 = dt.float32, dt.float32r, dt.bfloat16
    AF = mybir.ActivationFunctionType
    OP = mybir.AluOpType

    nc = bacc.Bacc("TRN2", target_bir_lowering=False, debug=False,
                   num_devices=NCORES)

    hsT = nc.dram_tensor("hidden_T", [H, BLOC], f32, kind="ExternalInput").ap()
    af = nc.dram_tensor("att_feats", [BLOC, N, H], f32r, kind="ExternalInput").ap()
    pa = nc.dram_tensor("p_att_feats", [BLOC, N, A], f32, kind="ExternalInput").ap()
    am = nc.dram_tensor("att_masks", [BLOC, N], f32, kind="ExternalInput").ap()
    whT = nc.dram_tensor("W_hT", [H, A], f32, kind="ExternalInput").ap()
    wab = nc.dram_tensor("W_alpha_b", [P, A], bf16, kind="ExternalInput").ap()
    out = nc.dram_tensor("att_out", [BLOC, H], f32, kind="ExternalOutput").ap()

    with tile.TileContext(nc) as tc:
        with (
            tc.tile_pool(name="consts", bufs=1) as consts,
            tc.tile_pool(name="patt", bufs=3) as patt_pool,
            tc.tile_pool(name="alpha", bufs=2) as alpha_pool,
            tc.tile_pool(name="afp", bufs=4) as af_pool,
            tc.tile_pool(name="small", bufs=2) as small,
            tc.tile_pool(name="psmisc", bufs=2, space="PSUM") as psmisc,
            tc.tile_pool(name="psatt", bufs=6, space="PSUM") as psatt,
        ):
            # ---------------- setup ----------------
            ones_col = consts.tile([P, 1], f32)
            nc.vector.memset(ones_col, 1.0)

            whT_sb = []
            for hc in range(H // P):  # 8 tiles [128h, 512a], contiguous rows
                t = consts.tile([P, A], f32, name=f"whT{hc}", tag=f"whT{hc}")
                nc.sync.dma_start(out=t, in_=whT[hc * P:(hc + 1) * P, :])
                whT_sb.append(t)
            hidT_sb = []
            for hc in range(H // P):  # 8 tiles [128h, 8b]
                t = consts.tile([P, BLOC], f32, name=f"hidT{hc}", tag=f"hidT{hc}")
                nc.sync.dma_start(out=t, in_=hsT[hc * P:(hc + 1) * P, :])
                hidT_sb.append(t)
            wa_bf = consts.tile([P, A], bf16)
            nc.sync.dma_start(out=wa_bf, in_=wab[:, :])

            # w_h = hidden @ W_h.T : [8, 512]
            wh_ps = psmisc.tile([BLOC, A], f32, tag="mm")
            for hc in range(H // P):
                nc.tensor.matmul(wh_ps, lhsT=hidT_sb[hc], rhs=whT_sb[hc],
                                 start=(hc == 0), stop=(hc == H // P - 1))
            whall_sb = consts.tile([BLOC, A], f32)
            nc.vector.tensor_copy(whall_sb, wh_ps)

            # per-batch w_h row broadcast to [128, 512] f32 via a DRAM
            # round-trip with a 0-stride partition AP (setup-only, ~2MB)
            whall_dram = nc.dram_tensor("whall_scratch", [BLOC, A], f32).ap()
            nc.sync.dma_start(out=whall_dram, in_=whall_sb)
            whb = []
            for b in range(BLOC):
                t = consts.tile([P, A], f32, name=f"whb{b}", tag=f"whb{b}")
                row = whall_dram[b:b + 1, :]
                src = bass.AP(tensor=row.tensor, offset=row.offset,
                              ap=[[0, P], row.ap[1]])
                nc.sync.dma_start(out=t, in_=src)
                whb.append(t)

            # ---------------- main loop (software-pipelined) ----------------
            # n = p*NT + c everywhere below.
            pa_r = [pa[b, :, :].rearrange("(p c) a -> p c a", c=NT)
                    for b in range(BLOC)]
            af_r = [af[b, :, :].rearrange("(p c) h -> p c h", c=NT)
                    for b in range(BLOC)]

            def patt_phase(b):
                scores = small.tile([P, NT], f32, tag="scores",
                                    name=f"scores{b}")
                for st in range(NT // PATT_SUP):  # 2 supertiles
                    pt = patt_pool.tile([P, PATT_SUP, A], f32, tag="patt",
                                        name=f"patt{b}_{st}")
                    nc.sync.dma_start(
                        out=pt,
                        in_=pa_r[b][:, st * PATT_SUP:(st + 1) * PATT_SUP, :],
                    )
                    whb_b = _free_bcast(bass, whb[b][:, :], PATT_SUP)
                    nc.vector.tensor_tensor(out=pt, in0=pt, in1=whb_b, op=OP.add)
                    ab = alpha_pool.tile([P, PATT_SUP, A], bf16, tag="alpha",
                                         name=f"alpha{b}_{st}")
                    nc.scalar.activation(ab, pt, AF.Tanh)
                    for c in range(PATT_SUP):
                        col = st * PATT_SUP + c
                        # out = (ab * 1) * wa ; accum_out = row-sum -> scores
                        nc.vector.scalar_tensor_tensor(
                            out=ab[:, c, :], in0=ab[:, c, :], scalar=1.0,
                            in1=wa_bf, op0=OP.mult, op1=OP.mult,
                            accum_out=scores[:, col:col + 1],
                        )

                masks = small.tile([P, NT], f32, tag="masks", name=f"masks{b}")
                nc.sync.dma_start(
                    out=masks, in_=am[b, :].rearrange("(p c) -> p c", c=NT))
                nc.vector.tensor_tensor(out=scores, in0=scores, in1=masks,
                                        op=OP.add)

                expt = small.tile([P, NT], f32r, tag="expt", name=f"expt{b}")
                rowsum = small.tile([P, 1], f32, tag="rowsum", name=f"rowsum{b}")
                nc.scalar.activation(expt, scores, AF.Exp, accum_out=rowsum)

                sum_ps = psmisc.tile([1, 1], f32, tag="mm", name=f"sum_ps{b}")
                nc.tensor.matmul(sum_ps, lhsT=rowsum, rhs=ones_col,
                                 start=True, stop=True)
                inv = small.tile([1, 1], f32, tag="inv", name=f"inv{b}")
                nc.vector.reciprocal(inv, sum_ps)
                return expt, inv

            def af_phase(b, expt, inv):
                att_lo = psatt.tile([1, A], f32, tag="att", name=f"attlo{b}")
                att_hi = psatt.tile([1, A], f32, tag="att", name=f"atthi{b}")
                for st2 in range(NT // AF_SUP):
                    aft = af_pool.tile([P, AF_SUP, H], f32r, tag="af",
                                       name=f"af{b}_{st2}")
                    nc.sync.dma_start(
                        out=aft,
                        in_=af_r[b][:, st2 * AF_SUP:(st2 + 1) * AF_SUP, :],
                    )
                    for c in range(AF_SUP):
                        t = st2 * AF_SUP + c
                        lhs = expt[:, t:t + 1]
                        nc.tensor.matmul(att_lo, lhsT=lhs,
                                         rhs=aft[:, c, 0:A],
                                         start=(t == 0), stop=(t == NT - 1))
                        nc.tensor.matmul(att_hi, lhsT=lhs,
                                         rhs=aft[:, c, A:H],
                                         start=(t == 0), stop=(t == NT - 1))

                att_row = small.tile([1, H], f32, tag="attrow",
                                     name=f"attrow{b}")
                nc.vector.tensor_scalar_mul(att_row[:, 0:A], att_lo, inv)
                nc.vector.tensor_scalar_mul(att_row[:, A:H], att_hi, inv)
                nc.sync.dma_start(out=out[b:b + 1, :], in_=att_row)

            state = {}
            for b in range(BLOC):
                state[b] = patt_phase(b)
                if b >= 1:
                    af_phase(b - 1, *state.pop(b - 1))
            af_phase(BLOC - 1, *state.pop(BLOC - 1))

    nc.compile()
    return nc


def _get_nc():
    if "nc" not in _NC_CACHE:
        _NC_CACHE["nc"] = _build_nc()
    return _NC_CACHE["nc"]


def kernel(hidden_states, att_feats, p_att_feats, att_masks, W_h, W_alpha):
    import ml_dtypes
    from concourse.bass_utils import run_bass_kernel_spmd

    nc = _get_nc()
    hidden_states = np.ascontiguousarray(hidden_states, dtype=np.float32)
    att_feats = np.ascontiguousarray(att_feats, dtype=np.float32)
    p_att_feats = np.ascontiguousarray(p_att_feats, dtype=np.float32)
    att_masks = np.ascontiguousarray(att_masks, dtype=np.float32)
    W_h = np.ascontiguousarray(W_h, dtype=np.float32)
    W_alpha = np.asarray(W_alpha, dtype=np.float32).reshape(1, A)

    whT = np.ascontiguousarray(W_h.T)                       # [H, A]
    wab = np.ascontiguousarray(
        np.broadcast_to(W_alpha, (P, A))).astype(ml_dtypes.bfloat16)

    in_maps = []
    for i in range(NCORES):
        s = slice(i * BLOC, (i + 1) * BLOC)
        in_maps.append({
            "hidden_T": np.ascontiguousarray(hidden_states[s].T),
            "att_feats": att_feats[s],
            "p_att_feats": p_att_feats[s],
            "att_masks": att_masks[s],
            "W_hT": whT,
            "W_alpha_b": wab,
        })

    global _LAST_IN_MAPS
    _LAST_IN_MAPS = in_maps
    res = run_bass_kernel_spmd(nc, in_maps, core_ids=list(range(NCORES)))
    return np.concatenate(
        [res.results[i]["att_out"] for i in range(NCORES)], axis=0
    ).astype(np.float32)


_LAST_IN_MAPS = None


# revision 16
# speedup vs baseline: 1.0990x; 1.0031x over previous
"""Bass/Tile TRN2 kernel for BasicAttention.

att = softmax(tanh(hidden @ W_h.T + p_att_feats) @ W_alpha + mask) @ att_feats

Shapes: B=64, N=2048, H=1024, A=512. Data-parallel over batch across 8
NeuronCores (8 batches per core); weights replicated; no collectives.

Layout: region index n maps to (partition p, column c) as n = p*16 + c so
every p_att/att_feats DMA is a long contiguous per-partition read and the
mask tile is a natural row-major reshape.

Per-core dataflow (memory-bound: ~96MB HBM reads/core):
  host: pass W_h.T, hidden.T and a pre-broadcast bf16 W_alpha (layout-only
        transforms) so no PE transposes are needed on device.
  setup: w_h = hidden @ W_h.T (PE) -> per-batch partition-broadcast of
         w_h rows via a DRAM round-trip 0-stride DMA.
  per batch b (software-pipelined, p_att phase leads att_feats phase):
    p_att stream [128,8,512]: DVE add (w_h bcast) -> ACT tanh (bf16)
      -> DVE scalar_tensor_tensor vs W_alpha (accum) -> scores[128,16]
    scores: + mask, ACT exp (accum rowsum, f32r out), PE total-sum,
      DVE reciprocal
    att_feats stream [128,2,1024] f32r: PE matmuls (attn col stationary)
      accumulating att[1,1024] in PSUM -> DVE scale by 1/sum -> out.
"""

import numpy as np

B, N, H, A = 64, 2048, 1024, 512
NCORES = 8
BLOC = B // NCORES  # batches per core

P = 128
NT = N // P            # 16 n-columns per partition
PATT_SUP = 8           # columns per p_att supertile (2 DMAs per batch)
AF_SUP = 4             # columns per att_feats supertile (4 DMAs per batch)

_NC_CACHE = {}


def _free_bcast(bass_mod, ap, repeat):
    """[P, F] AP -> [P, repeat, F] AP with 0-stride middle dim."""
    return bass_mod.AP(
        tensor=ap.tensor,
        offset=ap.offset,
        ap=[ap.ap[0], [0, repeat], *ap.ap[1:]],
    )


def _build_nc():
    import concourse.bass as bass
    import concourse.mybir as mybir
    import concourse.tile as tile
    from concourse import bacc

    dt = mybir.dt
    f32, f32r, bf16 = dt.float32, dt.float32r, dt.bfloat16
    AF = mybir.ActivationFunctionType
    OP = mybir.AluOpType

    nc = bacc.Bacc("TRN2", target_bir_lowering=False, debug=False,
                   num_devices=NCORES)

    hsT = nc.dram_tensor("hidden_T", [H, BLOC], f32, kind="ExternalInput").ap()
    af = nc.dram_tensor("att_feats", [BLOC, N, H], f32r, kind="ExternalInput").ap()
    pa = nc.dram_tensor("p_att_feats", [BLOC, N, A], f32, kind="ExternalInput").ap()
    am = nc.dram_tensor("att_masks", [BLOC, N], f32, kind="ExternalInput").ap()
    whT = nc.dram_tensor("W_hT", [H, A], f32, kind="ExternalInput").ap()
    wab = nc.dram_tensor("W_alpha_b", [P, A], bf16, kind="ExternalInput").ap()
    out = nc.dram_tensor("att_out", [BLOC, H], f32, kind="ExternalOutput").ap()

    with tile.TileContext(nc) as tc:
        with (
            tc.tile_pool(name="consts", bufs=1) as consts,
            tc.tile_pool(name="patt", bufs=4) as patt_pool,
            tc.tile_pool(name="alpha", bufs=3) as alpha_pool,
            tc.tile_pool(name="afp", bufs=3) as af_pool,
            tc.tile_pool(name="small", bufs=4) as small,
            tc.tile_pool(name="psmisc", bufs=2, space="PSUM") as psmisc,
            tc.tile_pool(name="psatt", bufs=6, space="PSUM") as psatt,
        ):
            # ---------------- setup ----------------
            ones_col = consts.tile([P, 1], f32)
            nc.vector.memset(ones_col, 1.0)

            whT_sb = []
            for hc in range(H // P):  # 8 tiles [128h, 512a], contiguous rows
                t = consts.tile([P, A], f32, name=f"whT{hc}", tag=f"whT{hc}")
                nc.sync.dma_start(out=t, in_=whT[hc * P:(hc + 1) * P, :])
                whT_sb.append(t)
            hidT_sb = []
            for hc in range(H // P):  # 8 tiles [128h, 8b]
                t = consts.tile([P, BLOC], f32, name=f"hidT{hc}", tag=f"hidT{hc}")
                nc.sync.dma_start(out=t, in_=hsT[hc * P:(hc + 1) * P, :])
                hidT_sb.append(t)
            wa_bf = consts.tile([P, A], bf16)
            nc.sync.dma_start(out=wa_bf, in_=wab[:, :])

            # w_h = hidden @ W_h.T : [8, 512]
            wh_ps = psmisc.tile([BLOC, A], f32, tag="mm")
            for hc in range(H // P):
                nc.tensor.matmul(wh_ps, lhsT=hidT_sb[hc], rhs=whT_sb[hc],
                                 start=(hc == 0), stop=(hc == H // P - 1))
            whall_sb = consts.tile([BLOC, A], f32)
            nc.vector.tensor_copy(whall_sb, wh_ps)

            # per-batch w_h row broadcast to [128, 512] f32 via a DRAM
            # round-trip with a 0-stride partition AP (setup-only, ~2MB)
            whall_dram = nc.dram_tensor("whall_scratch", [BLOC, A], f32).ap()
            nc.sync.dma_start(out=whall_dram, in_=whall_sb)
            whb = []
            for b in range(BLOC):
                t = consts.tile([P, A], f32, name=f"whb{b}", tag=f"whb{b}")
                row = whall_dram[b:b + 1, :]
                src = bass.AP(tensor=row.tensor, offset=row.offset,
                              ap=[[0, P], row.ap[1]])
                nc.sync.dma_start(out=t, in_=src)
                whb.append(t)

            # ---------------- main loop (software-pipelined) ----------------
            # n = p*NT + c everywhere below.
            pa_r = [pa[b, :, :].rearrange("(p c) a -> p c a", c=NT)
                    for b in range(BLOC)]
            af_r = [af[b, :, :].rearrange("(p c) h -> p c h", c=NT)
                    for b in range(BLOC)]

            def patt_phase(b):
                scores = small.tile([P, NT], f32, tag="scores",
                                    name=f"scores{b}")
                for st in range(NT // PATT_SUP):  # 2 supertiles
                    pt = patt_pool.tile([P, PATT_SUP, A], f32, tag="patt",
                                        name=f"patt{b}_{st}")
                    nc.sync.dma_start(
                        out=pt,
                        in_=pa_r[b][:, st * PATT_SUP:(st + 1) * PATT_SUP, :],
                    )
                    whb_b = _free_bcast(bass, whb[b][:, :], PATT_SUP)
                    nc.vector.tensor_tensor(out=pt, in0=pt, in1=whb_b, op=OP.add)
                    ab = alpha_pool.tile([P, PATT_SUP, A], bf16, tag="alpha",
                                         name=f"alpha{b}_{st}")
                    nc.scalar.activation(ab, pt, AF.Tanh)
                    for c in range(PATT_SUP):
                        col = st * PATT_SUP + c
                        # out = (ab * 1) * wa ; accum_out = row-sum -> scores
                        nc.vector.scalar_tensor_tensor(
                            out=ab[:, c, :], in0=ab[:, c, :], scalar=1.0,
                            in1=wa_bf, op0=OP.mult, op1=OP.mult,
                            accum_out=scores[:, col:col + 1],
                        )

                masks = small.tile([P, NT], f32, tag="masks", name=f"masks{b}")
                nc.sync.dma_start(
                    out=masks, in_=am[b, :].rearrange("(p c) -> p c", c=NT))
                nc.vector.tensor_tensor(out=scores, in0=scores, in1=masks,
                                        op=OP.add)

                expt = small.tile([P, NT], f32r, tag="expt", name=f"expt{b}")
                rowsum = small.tile([P, 1], f32, tag="rowsum", name=f"rowsum{b}")
                nc.scalar.activation(expt, scores, AF.Exp, accum_out=rowsum)

                sum_ps = psmisc.tile([1, 1], f32, tag="mm", name=f"sum_ps{b}")
                nc.tensor.matmul(sum_ps, lhsT=rowsum, rhs=ones_col,
                                 start=True, stop=True)
                inv = small.tile([1, 1], f32, tag="inv", name=f"inv{b}")
                nc.vector.reciprocal(inv, sum_ps)
                return expt, inv

            def af_phase(b, expt, inv):
                att_lo = psatt.tile([1, A], f32, tag="att", name=f"attlo{b}")
                att_hi = psatt.tile([1, A], f32, tag="att", name=f"atthi{b}")
                for st2 in range(NT // AF_SUP):
                    aft = af_pool.tile([P, AF_SUP, H], f32r, tag="af",
                                       name=f"af{b}_{st2}")
                    nc.sync.dma_start(
                        out=aft,
                        in_=af_r[b][:, st2 * AF_SUP:(st2 + 1) * AF_SUP, :],
                    )
                    for c in range(AF_SUP):
                        t = st2 * AF_SUP + c
                        lhs = expt[:, t:t + 1]
                        nc.tensor.matmul(att_lo, lhsT=lhs,
                                         rhs=aft[:, c, 0:A],
                                         start=(t == 0), stop=(t == NT - 1))
                        nc.tensor.matmul(att_hi, lhsT=lhs,
                                         rhs=aft[:, c, A:H],
                                         start=(t == 0), stop=(t == NT - 1))

                att_row = small.tile([1, H], f32, tag="attrow",
                                     name=f"attrow{b}")
                nc.vector.tensor_scalar_mul(att_row[:, 0:A], att_lo, inv)
                nc.vector.tensor_scalar_mul(att_row[:, A:H], att_hi, inv)
                nc.sync.dma_start(out=out[b:b + 1, :], in_=att_row)

            state = {}
            for b in range(BLOC):
                state[b] = patt_phase(b)
                if b >= 1:
                    af_phase(b - 1, *state.pop(b - 1))
            af_phase(BLOC - 1, *state.pop(BLOC - 1))

    nc.compile()
    return nc


def _get_nc():
    if "nc" not in _NC_CACHE:
        _NC_CACHE["nc"] = _build_nc()
    return _NC_CACHE["nc"]


def kernel(hidden_states, att_feats, p_att_feats, att_masks, W_h, W_alpha):
    import ml_dtypes
    from concourse.bass_utils import run_bass_kernel_spmd

    nc = _get_nc()
    hidden_states = np.ascontiguousarray(hidden_states, dtype=np.float32)
    att_feats = np.ascontiguousarray(att_feats, dtype=np.float32)
    p_att_feats = np.ascontiguousarray(p_att_feats, dtype=np.float32)
    att_masks = np.ascontiguousarray(att_masks, dtype=np.float32)
    W_h = np.ascontiguousarray(W_h, dtype=np.float32)
    W_alpha = np.asarray(W_alpha, dtype=np.float32).reshape(1, A)

    whT = np.ascontiguousarray(W_h.T)                       # [H, A]
    wab = np.ascontiguousarray(
        np.broadcast_to(W_alpha, (P, A))).astype(ml_dtypes.bfloat16)

    in_maps = []
    for i in range(NCORES):
        s = slice(i * BLOC, (i + 1) * BLOC)
        in_maps.append({
            "hidden_T": np.ascontiguousarray(hidden_states[s].T),
            "att_feats": att_feats[s],
            "p_att_feats": p_att_feats[s],
            "att_masks": att_masks[s],
            "W_hT": whT,
            "W_alpha_b": wab,
        })

    global _LAST_IN_MAPS
    _LAST_IN_MAPS = in_maps
    res = run_bass_kernel_spmd(nc, in_maps, core_ids=list(range(NCORES)))
    return np.concatenate(
        [res.results[i]["att_out"] for i in range(NCORES)], axis=0
    ).astype(np.float32)


_LAST_IN_MAPS = None
